# revision 25
# baseline (speedup 1.0000x reference)
"""Trainium2 Bass kernel for nn_CrossAttentionMatcher.

Reference math (B=2, L1=L2=512, H=256, NH=8, DH=32):
  q = e1 @ Wq.T + bq ; k = e2 @ Wk.T + bk
  logits[b,h,i,j] = (q.head[h][i] . k.head[h][j]) / sqrt(DH)
  attn_weights   = softmax(logits, -1).mean(heads)
  u = e1 @ W1a.T + b1 ; v = e2 @ W1b.T
  match_scores[b,i,j] = sigmoid( sum_h W2[h]*relu(u[b,i,h]+v[b,j,h]) + b2 )

Sharding: the 1024 (b,i) rows split into 8 slices of 128 rows, one per
NeuronCore (cores 0-3 -> batch 0, 4-7 -> batch 1). Each core holds full
embed2 for its batch plus all params, and computes its [128, 512] slice of
both outputs.

Per-core device program (Tile framework), all matmuls bf16:
  - prep matmuls produce transposed activations (contraction on partitions):
    qT,kT (bias folded, q pre-scaled), u=(h1T+b1) fp32, v=h2T bf16
  - attention: 8 K=32 matmuls -> PSUM logits; ACT exp with accum_out giving
    row sums for free; DVE reciprocal + ACT per-head scale + DVE strided
    head-sum
  - pairwise MLP (the bulk): for each of 128 i-rows x 2 h-blocks,
    R = relu(v + u[:,i]) as one DVE tensor_scalar (bf16 4x mode) or ACT
    activation (load-balance offload). R is the stationary matmul operand
    (4 j-blocks of [128,128]); w2 streams as N=1 moving operand; outputs
    scoresT[j, (jb,i)] accumulate into ONE PSUM bank as a single
    accumulation group (start only on the very first matmul -- start=True
    clears has_written for the whole bank; per-element has_written gives
    overwrite-on-first-touch then accumulate). Host transposes back.
  - sigmoid computed as exp + 1/(1+e) so every ACT op stays in the
    exp_and_others table set (one table load per kernel).

build_program(repeat=N) wraps the compute body in a tc.For_i loop for
wall-clock slope timing (the graded path uses repeat=1).
"""

import os
import numpy as np
import ml_dtypes

import concourse.bass as bass
import concourse.bacc as bacc
import concourse.mybir as mybir
from concourse import tile
from concourse.bass_utils import run_bass_kernel_spmd

F32 = mybir.dt.float32
BF16 = mybir.dt.bfloat16
AF = mybir.ActivationFunctionType
ALU = mybir.AluOpType
ET = mybir.EngineType

H = 256
NH = 8
DH = 32
B = 2
L1 = 512
L2 = 512
NCORES = 8
SL = 128          # L1-rows per core
CB = 2            # contraction blocks (256 = 2*128)
HB = 2            # hidden blocks (256 = 2*128)
JB = 4            # j blocks of 128
QSCALE = 1.0 / np.sqrt(DH)

# pairwise R-tile engine assignment by k%32 (GPSIMD measured ~8us/op on HW --
# never route compute there)
R_ACT_SLOTS = frozenset(
    int(x) for x in os.environ.get(
        "KERNEL_R_ACT", "1,6,11,16,21,27").split(",") if x)


def build_program(repeat=1):
    nc = bacc.Bacc(
        "TRN2",
        target_bir_lowering=False,
        debug=False,
    )

    # ---- DRAM I/O (per-core shards; names are the in_map keys) ----
    e1t_d = nc.dram_tensor("e1t", [CB, 128, SL], BF16, kind="ExternalInput")
    e2t_d = nc.dram_tensor("e2t", [CB, 128, L2], BF16, kind="ExternalInput")
    wqt_d = nc.dram_tensor("wqt", [CB, 128, H], BF16, kind="ExternalInput")
    wkt_d = nc.dram_tensor("wkt", [CB, 128, H], BF16, kind="ExternalInput")
    w1at_d = nc.dram_tensor("w1at", [CB, 128, H], BF16, kind="ExternalInput")
    w1bt_d = nc.dram_tensor("w1bt", [CB, 128, H], BF16, kind="ExternalInput")
    bq_d = nc.dram_tensor("bq", [128, HB], F32, kind="ExternalInput")
    bk_d = nc.dram_tensor("bk", [128, HB], F32, kind="ExternalInput")
    b1_d = nc.dram_tensor("b1c", [128, HB], F32, kind="ExternalInput")
    w2_d = nc.dram_tensor("w2c", [128, HB], BF16, kind="ExternalInput")
    b2n_d = nc.dram_tensor("b2neg", [128, 1], F32, kind="ExternalInput")
    attn_d = nc.dram_tensor("attn", [SL, L2], BF16, kind="ExternalOutput")
    scores_d = nc.dram_tensor("scoresT", [128, JB, SL], BF16,
                              kind="ExternalOutput")

    with tile.TileContext(nc) as tc:
        with (
            tc.tile_pool(name="const", bufs=1) as const,
            tc.tile_pool(name="work", bufs=1) as work,
            tc.tile_pool(name="rpool", bufs=64) as rpool,
            tc.tile_pool(name="pps", bufs=2, space="PSUM") as pps,
            tc.tile_pool(name="ppb", bufs=4, space="PSUM") as ppb,
            tc.tile_pool(name="pscp", bufs=1, space="PSUM") as pscp,
        ):
            # ---- loads (outside the repeat loop) ----
            E1T = const.tile([128, CB, SL], BF16, tag="e1t")
            E2T = const.tile([128, CB, L2], BF16, tag="e2t")
            WQ = const.tile([128, CB, H], BF16, tag="wqt")
            WK = const.tile([128, CB, H], BF16, tag="wkt")
            W1A = const.tile([128, CB, H], BF16, tag="w1at")
            W1B = const.tile([128, CB, H], BF16, tag="w1bt")
            BQ = const.tile([128, HB], F32, tag="bq")
            BK = const.tile([128, HB], F32, tag="bk")
            B1 = const.tile([128, HB], F32, tag="b1")
            W2C = const.tile([128, HB], BF16, tag="w2c")
            B2N = const.tile([128, 1], F32, tag="b2n")
            # u/v-path tensors first so prep matmuls start ASAP; spread across
            # the two HWDGE queues (SP + ACT)
            for cb in range(CB):
                nc.sync.dma_start(E2T[:, cb, :], e2t_d[cb])
                nc.scalar.dma_start(E1T[:, cb, :], e1t_d[cb])
                nc.scalar.dma_start(W1A[:, cb, :], w1at_d[cb])
                nc.sync.dma_start(W1B[:, cb, :], w1bt_d[cb])
            nc.scalar.dma_start(B1[:], b1_d[:])
            nc.scalar.dma_start(W2C[:], w2_d[:])
            for cb in range(CB):
                nc.scalar.dma_start(WQ[:, cb, :], wqt_d[cb])
                nc.sync.dma_start(WK[:, cb, :], wkt_d[cb])
            nc.scalar.dma_start(BQ[:], bq_d[:])
            nc.sync.dma_start(BK[:], bk_d[:])
            nc.sync.dma_start(B2N[:], b2n_d[:])

            U = work.tile([128, HB, SL], F32, tag="u")
            V = work.tile([128, HB, L2], BF16, tag="v")
            QT = work.tile([128, HB, SL], BF16, tag="qt")
            KT = work.tile([128, HB, L2], BF16, tag="kt")
            EXPS = work.tile([128, NH, L2], BF16, tag="exps")
            S8 = work.tile([128, NH], F32, tag="s8")
            C8 = work.tile([128, NH], F32, tag="c8")
            T4 = work.tile([128, 4, L2], BF16, tag="t4")
            T2 = work.tile([128, 2, L2], BF16, tag="t2")
            ATTN = work.tile([128, L2], BF16, tag="attn")
            SCE = work.tile([128, JB, SL], F32, tag="sce")
            SCORES = work.tile([128, JB, SL], BF16, tag="scores")

            def emit_body():
                # ---- prep matmuls for the pairwise MLP (u, v) ----
                for m in range(HB):
                    hs = slice(m * 128, (m + 1) * 128)
                    pu = pps.tile([128, SL], F32, tag="pp_small")
                    for cb in range(CB):
                        nc.tensor.matmul(pu[:], W1A[:, cb, hs], E1T[:, cb, :],
                                         start=(cb == 0), stop=(cb == 1))
                    nc.scalar.activation(U[:, m, :], pu[:], AF.Identity,
                                         bias=B1[:, m:m + 1])
                    pv = ppb.tile([128, L2], F32, tag="pp_big")
                    for cb in range(CB):
                        nc.tensor.matmul(pv[:], W1B[:, cb, hs], E2T[:, cb, :],
                                         start=(cb == 0), stop=(cb == 1))
                    nc.scalar.activation(V[:, m, :], pv[:], AF.Copy)

                psc = pscp.tile([128, JB, SL], F32, tag="psc")

                # single accumulation group for the whole bank (see docstring)
                nmm = [0]
                nmm_total = SL * HB * JB

                def pair_step(i, hb):
                    r = rpool.tile([128, L2], BF16, tag="r")
                    k = i * HB + hb
                    if k % 32 in R_ACT_SLOTS:
                        nc.scalar.activation(r[:], V[:, hb, :], AF.Relu,
                                             bias=U[:, hb, i:i + 1])
                    else:
                        nc.vector.tensor_scalar(r[:], V[:, hb, :],
                                                U[:, hb, i:i + 1], 0.0,
                                                ALU.add, ALU.max)
                    for jb in range(JB):
                        nc.tensor.matmul(psc[:, jb, i:i + 1],
                                         r[:, 128 * jb:128 * (jb + 1)],
                                         W2C[:, hb:hb + 1],
                                         start=(nmm[0] == 0),
                                         stop=(nmm[0] == nmm_total - 1),
                                         skip_group_check=True)
                        nmm[0] += 1

                pair = [(i, hb) for i in range(SL) for hb in range(HB)]
                pos = [0]

                def emit_pair(n):
                    for i, hb in pair[pos[0]:pos[0] + n]:
                        pair_step(i, hb)
                    pos[0] += n

                emit_pair(32)

                # ---- attention: qT, kT ----
                for m in range(HB):
                    hs = slice(m * 128, (m + 1) * 128)
                    pq = pps.tile([128, SL], F32, tag="pp_small")
                    for cb in range(CB):
                        nc.tensor.matmul(pq[:], WQ[:, cb, hs], E1T[:, cb, :],
                                         start=(cb == 0), stop=(cb == 1))
                    # host pre-scales bq by QSCALE: qT = pq*QSCALE + bq*QSCALE
                    nc.scalar.activation(QT[:, m, :], pq[:], AF.Identity,
                                         bias=BQ[:, m:m + 1], scale=QSCALE)
                    pk = ppb.tile([128, L2], F32, tag="pp_big")
                    for cb in range(CB):
                        nc.tensor.matmul(pk[:], WK[:, cb, hs], E2T[:, cb, :],
                                         start=(cb == 0), stop=(cb == 1))
                    nc.scalar.activation(KT[:, m, :], pk[:], AF.Identity,
                                         bias=BK[:, m:m + 1])

                emit_pair(16)

                # ---- logits + exp (row-sums free via accum_out) ----
                # 4 heads per group use distinct PE row-groups and distinct
                # PSUM banks; issued back-to-back they overlap in the array
                for g in range(2):
                    pls = []
                    for j in range(4):
                        hd = g * 4 + j
                        m = hd // 4
                        ps = slice(32 * j, 32 * j + 32)
                        pl = ppb.tile([128, L2], F32, tag="pp_big",
                                      name=f"pl_{hd}")
                        nc.tensor.matmul(pl[:], QT[ps, m, :], KT[ps, m, :],
                                         start=True, stop=True,
                                         tile_position=(32 * j, 0))
                        pls.append(pl)
                    for j in range(4):
                        hd = g * 4 + j
                        nc.scalar.activation(EXPS[:, hd, :], pls[j][:], AF.Exp,
                                             accum_out=S8[:, hd:hd + 1])
                        emit_pair(8)

                # ---- softmax tail ----
                nc.vector.reciprocal(C8[:], S8[:])
                nc.vector.tensor_scalar(C8[:], C8[:], 1.0 / NH, None, ALU.mult)
                emit_pair(8)
                for hd in range(NH):
                    nc.scalar.activation(EXPS[:, hd, :], EXPS[:, hd, :],
                                         AF.Copy, scale=C8[:, hd:hd + 1])
                    emit_pair(4)
                emit_pair(8)
                # head-sum: bf16 pairwise tree (TT 2x mode beats 1x reduce);
                # attn values are <=1 and the tolerance is rel ~1e-2, so bf16
                # accumulation of 8 terms is fine
                with nc.allow_low_precision(reason="attn head-mean in bf16"):
                    for p in range(4):
                        nc.vector.tensor_tensor(T4[:, p, :], EXPS[:, 2 * p, :],
                                                EXPS[:, 2 * p + 1, :], ALU.add)
                        emit_pair(2)
                    for p in range(2):
                        nc.vector.tensor_tensor(T2[:, p, :], T4[:, 2 * p, :],
                                                T4[:, 2 * p + 1, :], ALU.add)
                        emit_pair(2)
                    nc.vector.tensor_tensor(ATTN[:], T2[:, 0, :], T2[:, 1, :],
                                            ALU.add)
                nc.sync.dma_start(attn_d[:], ATTN[:])

                emit_pair(len(pair) - pos[0])

                # sigmoid(x+b2) = 1/(1+exp(-x-b2)); Exp keeps the ACT table
                # set unchanged (Sigmoid lives in a different set)
                nc.scalar.activation(SCE[:], psc[:], AF.Exp,
                                     bias=B2N[:, 0:1], scale=-1.0)
                with nc.allow_low_precision(reason="sigmoid output in bf16"):
                    nc.vector.tensor_scalar(SCE[:], SCE[:], 1.0, None, ALU.add)
                    nc.vector.reciprocal(SCORES[:], SCE[:])
                # split the store across both HWDGE queues
                nc.sync.dma_start(scores_d[:, :2, :], SCORES[:, :2, :])
                nc.scalar.dma_start(scores_d[:, 2:, :], SCORES[:, 2:, :])

            if repeat == 1:
                emit_body()
            else:
                with tc.For_i(0, repeat, 1,
                              hint_engines=(ET.PE, ET.DVE, ET.Activation)):
                    emit_body()

    nc.compile()
    return nc


_NC_CACHE = {}


def _get_program(repeat=1):
    if repeat not in _NC_CACHE:
        _NC_CACHE[repeat] = build_program(repeat)
    return _NC_CACHE[repeat]


def make_in_maps(embed1, embed2, in_proj_w, in_proj_b, W1, b1, W2, b2):
    bf = ml_dtypes.bfloat16
    f32 = np.float32

    def blkT(a):  # [n, c] -> transposed + contraction-blocked [CB, 128, n]
        return np.ascontiguousarray(
            a.astype(f32).T.reshape(CB, 128, -1)).astype(bf)

    wqt = blkT(in_proj_w[:H])
    wkt = blkT(in_proj_w[H:2 * H])
    w1at = blkT(W1[:, :H])       # [h, c] rows x contraction cols
    w1bt = blkT(W1[:, H:])
    # pre-scaled: device computes qT = (pq + bq)*QSCALE as pq*QSCALE + this
    bq = np.ascontiguousarray(
        in_proj_b[:H].reshape(HB, 128).T * QSCALE).astype(f32)
    bk = np.ascontiguousarray(in_proj_b[H:2 * H].reshape(HB, 128).T).astype(f32)
    b1c = np.ascontiguousarray(b1.reshape(HB, 128).T).astype(f32)
    w2c = np.ascontiguousarray(W2[0].reshape(HB, 128).T).astype(bf)
    b2neg = np.full((128, 1), -b2[0], dtype=f32)

    in_maps = []
    for c in range(NCORES):
        b, s = divmod(c, NCORES // B)
        r0 = s * SL
        in_maps.append({
            "e1t": blkT(embed1[b, r0:r0 + SL]),
            "e2t": blkT(embed2[b]),
            "wqt": wqt, "wkt": wkt, "w1at": w1at, "w1bt": w1bt,
            "bq": bq, "bk": bk, "b1c": b1c, "w2c": w2c, "b2neg": b2neg,
        })
    return in_maps


def _assemble(results):
    attn = np.empty((B, L1, L2), dtype=np.float32)
    scores = np.empty((B, L1, L2), dtype=np.float32)
    for c in range(NCORES):
        b, s = divmod(c, NCORES // B)
        r0 = s * SL
        attn[b, r0:r0 + SL] = results[c]["attn"].astype(np.float32)
        # scoresT is [j_in, jb, i] -> [i, jb*128 + j_in]
        st = results[c]["scoresT"].astype(np.float32)
        scores[b, r0:r0 + SL] = np.transpose(st, (2, 1, 0)).reshape(SL, L2)
    return attn, scores


def kernel(embed1, embed2, in_proj_w, in_proj_b, W1, b1, W2, b2):
    embed1 = np.asarray(embed1, dtype=np.float32)
    embed2 = np.asarray(embed2, dtype=np.float32)
    in_proj_w = np.asarray(in_proj_w, dtype=np.float32)
    in_proj_b = np.asarray(in_proj_b, dtype=np.float32)
    W1 = np.asarray(W1, dtype=np.float32)
    b1 = np.asarray(b1, dtype=np.float32)
    W2 = np.asarray(W2, dtype=np.float32)
    b2 = np.asarray(b2, dtype=np.float32)

    nc = _get_program(int(os.environ.get("KERNEL_REPEAT", "1")))
    in_maps = make_in_maps(embed1, embed2, in_proj_w, in_proj_b, W1, b1, W2, b2)
    res = run_bass_kernel_spmd(nc, in_maps, list(range(NCORES)))
    return _assemble(res.results)


# revision 26
# speedup vs baseline: 2.4266x; 2.4266x over previous
"""Trainium2 Bass kernel for nn_CrossAttentionMatcher.

Reference math (B=2, L1=L2=512, H=256, NH=8, DH=32):
  q = e1 @ Wq.T + bq ; k = e2 @ Wk.T + bk
  logits[b,h,i,j] = (q.head[h][i] . k.head[h][j]) / sqrt(DH)
  attn_weights   = softmax(logits, -1).mean(heads)
  u = e1 @ W1a.T + b1 ; v = e2 @ W1b.T
  match_scores[b,i,j] = sigmoid( sum_h W2[h]*relu(u[b,i,h]+v[b,j,h]) + b2 )

Sharding: the 1024 (b,i) rows split into 8 slices of 128 rows, one per
NeuronCore (cores 0-3 -> batch 0, 4-7 -> batch 1). Each core holds full
embed2 for its batch plus all params, and computes its [128, 512] slice of
both outputs.

Per-core device program (Tile framework), all matmuls bf16:
  - prep matmuls produce transposed activations (contraction on partitions):
    qT,kT (bias folded, q pre-scaled), u=(h1T+b1) fp32, v=h2T bf16
  - attention: 8 K=32 matmuls -> PSUM logits; ACT exp with accum_out giving
    row sums for free; DVE reciprocal + ACT per-head scale + DVE strided
    head-sum
  - pairwise MLP (the bulk): for each of 128 i-rows x 2 h-blocks,
    R = relu(v + u[:,i]) as one DVE tensor_scalar (bf16 4x mode) or ACT
    activation (load-balance offload). R is the stationary matmul operand
    (4 j-blocks of [128,128]); w2 streams as N=1 moving operand; outputs
    scoresT[j, (jb,i)] accumulate into ONE PSUM bank as a single
    accumulation group (start only on the very first matmul -- start=True
    clears has_written for the whole bank; per-element has_written gives
    overwrite-on-first-touch then accumulate). Host transposes back.
  - sigmoid computed as exp + 1/(1+e) so every ACT op stays in the
    exp_and_others table set (one table load per kernel).

build_program(repeat=N) wraps the compute body in a tc.For_i loop for
wall-clock slope timing (the graded path uses repeat=1).
"""

import os
import numpy as np
import ml_dtypes

import concourse.bass as bass
import concourse.bacc as bacc
import concourse.mybir as mybir
from concourse import tile
from concourse.bass_utils import run_bass_kernel_spmd

F32 = mybir.dt.float32
BF16 = mybir.dt.bfloat16
AF = mybir.ActivationFunctionType
ALU = mybir.AluOpType
ET = mybir.EngineType

H = 256
NH = 8
DH = 32
B = 2
L1 = 512
L2 = 512
NCORES = 8
SL = 128          # L1-rows per core
CB = 2            # contraction blocks (256 = 2*128)
HB = 2            # hidden blocks (256 = 2*128)
JB = 4            # j blocks of 128
QSCALE = 1.0 / np.sqrt(DH)

# pairwise R-tile engine assignment by k%32 (GPSIMD measured ~8us/op on HW --
# never route compute there)
R_ACT_SLOTS = frozenset(
    int(x) for x in os.environ.get(
        "KERNEL_R_ACT", "1,6,11,16,21,26,30").split(",") if x)


def build_program(repeat=1):
    nc = bacc.Bacc(
        "TRN2",
        target_bir_lowering=False,
        debug=False,
    )

    # ---- DRAM I/O (per-core shards; names are the in_map keys) ----
    e1t_d = nc.dram_tensor("e1t", [CB, 128, SL], BF16, kind="ExternalInput")
    e2t_d = nc.dram_tensor("e2t", [CB, 128, L2], BF16, kind="ExternalInput")
    wqt_d = nc.dram_tensor("wqt", [CB, 128, H], BF16, kind="ExternalInput")
    wkt_d = nc.dram_tensor("wkt", [CB, 128, H], BF16, kind="ExternalInput")
    w1at_d = nc.dram_tensor("w1at", [CB, 128, H], BF16, kind="ExternalInput")
    w1bt_d = nc.dram_tensor("w1bt", [CB, 128, H], BF16, kind="ExternalInput")
    bq_d = nc.dram_tensor("bq", [128, HB], F32, kind="ExternalInput")
    bk_d = nc.dram_tensor("bk", [128, HB], F32, kind="ExternalInput")
    b1_d = nc.dram_tensor("b1c", [128, HB], F32, kind="ExternalInput")
    w2_d = nc.dram_tensor("w2c", [128, HB], BF16, kind="ExternalInput")
    b2n_d = nc.dram_tensor("b2neg", [128, 1], F32, kind="ExternalInput")
    attn_d = nc.dram_tensor("attn", [SL, L2], F32, kind="ExternalOutput")
    scores_d = nc.dram_tensor("scoresT", [128, JB, SL], F32,
                              kind="ExternalOutput")

    with tile.TileContext(nc) as tc:
        with (
            tc.tile_pool(name="const", bufs=1) as const,
            tc.tile_pool(name="work", bufs=1) as work,
            tc.tile_pool(name="rpool", bufs=64) as rpool,
            tc.tile_pool(name="pps", bufs=2, space="PSUM") as pps,
            tc.tile_pool(name="ppb", bufs=4, space="PSUM") as ppb,
            tc.tile_pool(name="pscp", bufs=1, space="PSUM") as pscp,
        ):
            # ---- loads (outside the repeat loop) ----
            E1T = const.tile([128, CB, SL], BF16, tag="e1t")
            E2T = const.tile([128, CB, L2], BF16, tag="e2t")
            WQ = const.tile([128, CB, H], BF16, tag="wqt")
            WK = const.tile([128, CB, H], BF16, tag="wkt")
            W1A = const.tile([128, CB, H], BF16, tag="w1at")
            W1B = const.tile([128, CB, H], BF16, tag="w1bt")
            BQ = const.tile([128, HB], F32, tag="bq")
            BK = const.tile([128, HB], F32, tag="bk")
            B1 = const.tile([128, HB], F32, tag="b1")
            W2C = const.tile([128, HB], BF16, tag="w2c")
            B2N = const.tile([128, 1], F32, tag="b2n")
            # u/v-path tensors first so prep matmuls start ASAP; spread across
            # the two HWDGE queues (SP + ACT)
            for cb in range(CB):
                nc.sync.dma_start(E2T[:, cb, :], e2t_d[cb])
                nc.scalar.dma_start(E1T[:, cb, :], e1t_d[cb])
                nc.scalar.dma_start(W1A[:, cb, :], w1at_d[cb])
                nc.sync.dma_start(W1B[:, cb, :], w1bt_d[cb])
            nc.scalar.dma_start(B1[:], b1_d[:])
            nc.scalar.dma_start(W2C[:], w2_d[:])
            for cb in range(CB):
                nc.scalar.dma_start(WQ[:, cb, :], wqt_d[cb])
                nc.sync.dma_start(WK[:, cb, :], wkt_d[cb])
            nc.scalar.dma_start(BQ[:], bq_d[:])
            nc.sync.dma_start(BK[:], bk_d[:])
            nc.sync.dma_start(B2N[:], b2n_d[:])

            U = work.tile([128, HB, SL], F32, tag="u")
            V = work.tile([128, HB, L2], BF16, tag="v")
            QT = work.tile([128, HB, SL], BF16, tag="qt")
            KT = work.tile([128, HB, L2], BF16, tag="kt")
            EXPS = work.tile([128, NH, L2], F32, tag="exps")
            S8 = work.tile([128, NH], F32, tag="s8")
            C8 = work.tile([128, NH], F32, tag="c8")
            ATTN = work.tile([128, L2], F32, tag="attn")
            SCE = work.tile([128, JB, SL], F32, tag="sce")
            SCORES = work.tile([128, JB, SL], F32, tag="scores")

            def emit_body():
                # ---- prep matmuls for the pairwise MLP (u, v) ----
                for m in range(HB):
                    hs = slice(m * 128, (m + 1) * 128)
                    pu = pps.tile([128, SL], F32, tag="pp_small")
                    for cb in range(CB):
                        nc.tensor.matmul(pu[:], W1A[:, cb, hs], E1T[:, cb, :],
                                         start=(cb == 0), stop=(cb == 1))
                    nc.vector.tensor_scalar(U[:, m, :], pu[:], B1[:, m:m + 1],
                                            None, ALU.add)
                    pv = ppb.tile([128, L2], F32, tag="pp_big")
                    for cb in range(CB):
                        nc.tensor.matmul(pv[:], W1B[:, cb, hs], E2T[:, cb, :],
                                         start=(cb == 0), stop=(cb == 1))
                    nc.vector.tensor_copy(V[:, m, :], pv[:])

                psc = pscp.tile([128, JB, SL], F32, tag="psc")

                # single accumulation group for the whole bank (see docstring)
                nmm = [0]
                nmm_total = SL * HB * JB

                def pair_step(i, hb):
                    r = rpool.tile([128, L2], BF16, tag="r")
                    k = i * HB + hb
                    if k % 32 in R_ACT_SLOTS:
                        nc.scalar.activation(r[:], V[:, hb, :], AF.Relu,
                                             bias=U[:, hb, i:i + 1])
                    else:
                        nc.vector.tensor_scalar(r[:], V[:, hb, :],
                                                U[:, hb, i:i + 1], 0.0,
                                                ALU.add, ALU.max)
                    for jb in range(JB):
                        nc.tensor.matmul(psc[:, jb, i:i + 1],
                                         r[:, 128 * jb:128 * (jb + 1)],
                                         W2C[:, hb:hb + 1],
                                         start=(nmm[0] == 0),
                                         stop=(nmm[0] == nmm_total - 1),
                                         skip_group_check=True)
                        nmm[0] += 1

                pair = [(i, hb) for i in range(SL) for hb in range(HB)]
                pos = [0]

                def emit_pair(n):
                    for i, hb in pair[pos[0]:pos[0] + n]:
                        pair_step(i, hb)
                    pos[0] += n

                emit_pair(32)

                # ---- attention: qT, kT ----
                for m in range(HB):
                    hs = slice(m * 128, (m + 1) * 128)
                    pq = pps.tile([128, SL], F32, tag="pp_small")
                    for cb in range(CB):
                        nc.tensor.matmul(pq[:], WQ[:, cb, hs], E1T[:, cb, :],
                                         start=(cb == 0), stop=(cb == 1))
                    # host pre-scales bq by QSCALE: qT = pq*QSCALE + bq*QSCALE
                    nc.vector.tensor_scalar(QT[:, m, :], pq[:], QSCALE,
                                            BQ[:, m:m + 1], ALU.mult, ALU.add)
                    pk = ppb.tile([128, L2], F32, tag="pp_big")
                    for cb in range(CB):
                        nc.tensor.matmul(pk[:], WK[:, cb, hs], E2T[:, cb, :],
                                         start=(cb == 0), stop=(cb == 1))
                    nc.vector.tensor_scalar(KT[:, m, :], pk[:], BK[:, m:m + 1],
                                            None, ALU.add)

                emit_pair(16)

                # ---- logits + exp (row-sums free via accum_out) ----
                # 4 heads per group use distinct PE row-groups and distinct
                # PSUM banks; issued back-to-back they overlap in the array
                for g in range(2):
                    pls = []
                    for j in range(4):
                        hd = g * 4 + j
                        m = hd // 4
                        ps = slice(32 * j, 32 * j + 32)
                        pl = ppb.tile([128, L2], F32, tag="pp_big",
                                      name=f"pl_{hd}")
                        nc.tensor.matmul(pl[:], QT[ps, m, :], KT[ps, m, :],
                                         start=True, stop=True,
                                         tile_position=(32 * j, 0))
                        pls.append(pl)
                    for j in range(4):
                        hd = g * 4 + j
                        nc.scalar.activation(EXPS[:, hd, :], pls[j][:], AF.Exp,
                                             accum_out=S8[:, hd:hd + 1])
                        emit_pair(8)

                # ---- softmax tail ----
                nc.vector.reciprocal(C8[:], S8[:])
                nc.vector.tensor_scalar(C8[:], C8[:], 1.0 / NH, None, ALU.mult)
                emit_pair(8)
                for hd in range(NH):
                    nc.scalar.activation(EXPS[:, hd, :], EXPS[:, hd, :],
                                         AF.Copy, scale=C8[:, hd:hd + 1])
                    emit_pair(4)
                emit_pair(16)
                # head-sum: strided view [128, j, h], reduce innermost h
                nc.vector.tensor_reduce(ATTN[:],
                                        EXPS[:].rearrange("p h j -> p j h"),
                                        axis=mybir.AxisListType.X, op=ALU.add)
                nc.sync.dma_start(attn_d[:], ATTN[:])

                emit_pair(len(pair) - pos[0])

                # sigmoid(x+b2) = 1/(1+exp(-x-b2)); Exp keeps the ACT table
                # set unchanged (Sigmoid lives in a different set)
                nc.scalar.activation(SCE[:], psc[:], AF.Exp,
                                     bias=B2N[:, 0:1], scale=-1.0)
                nc.vector.tensor_scalar(SCORES[:], SCE[:], 1.0, None, ALU.add)
                nc.vector.reciprocal(SCORES[:], SCORES[:])
                # split the store across both HWDGE queues
                nc.sync.dma_start(scores_d[:, :2, :], SCORES[:, :2, :])
                nc.scalar.dma_start(scores_d[:, 2:, :], SCORES[:, 2:, :])

            if repeat == 1:
                emit_body()
            else:
                with tc.For_i(0, repeat, 1,
                              hint_engines=(ET.PE, ET.DVE, ET.Activation)):
                    emit_body()

    nc.compile()
    return nc


_NC_CACHE = {}


def _get_program(repeat=1):
    if repeat not in _NC_CACHE:
        _NC_CACHE[repeat] = build_program(repeat)
    return _NC_CACHE[repeat]


def make_in_maps(embed1, embed2, in_proj_w, in_proj_b, W1, b1, W2, b2):
    bf = ml_dtypes.bfloat16
    f32 = np.float32

    def blkT(a):  # [n, c] -> transposed + contraction-blocked [CB, 128, n]
        return np.ascontiguousarray(
            a.astype(f32).T.reshape(CB, 128, -1)).astype(bf)

    wqt = blkT(in_proj_w[:H])
    wkt = blkT(in_proj_w[H:2 * H])
    w1at = blkT(W1[:, :H])       # [h, c] rows x contraction cols
    w1bt = blkT(W1[:, H:])
    # pre-scaled: device computes qT = (pq + bq)*QSCALE as pq*QSCALE + this
    bq = np.ascontiguousarray(
        in_proj_b[:H].reshape(HB, 128).T * QSCALE).astype(f32)
    bk = np.ascontiguousarray(in_proj_b[H:2 * H].reshape(HB, 128).T).astype(f32)
    b1c = np.ascontiguousarray(b1.reshape(HB, 128).T).astype(f32)
    w2c = np.ascontiguousarray(W2[0].reshape(HB, 128).T).astype(bf)
    b2neg = np.full((128, 1), -b2[0], dtype=f32)

    in_maps = []
    for c in range(NCORES):
        b, s = divmod(c, NCORES // B)
        r0 = s * SL
        in_maps.append({
            "e1t": blkT(embed1[b, r0:r0 + SL]),
            "e2t": blkT(embed2[b]),
            "wqt": wqt, "wkt": wkt, "w1at": w1at, "w1bt": w1bt,
            "bq": bq, "bk": bk, "b1c": b1c, "w2c": w2c, "b2neg": b2neg,
        })
    return in_maps


def _assemble(results):
    attn = np.empty((B, L1, L2), dtype=np.float32)
    scores = np.empty((B, L1, L2), dtype=np.float32)
    for c in range(NCORES):
        b, s = divmod(c, NCORES // B)
        r0 = s * SL
        attn[b, r0:r0 + SL] = results[c]["attn"].astype(np.float32)
        # scoresT is [j_in, jb, i] -> [i, jb*128 + j_in]
        st = results[c]["scoresT"].astype(np.float32)
        scores[b, r0:r0 + SL] = np.transpose(st, (2, 1, 0)).reshape(SL, L2)
    return attn, scores


def kernel(embed1, embed2, in_proj_w, in_proj_b, W1, b1, W2, b2):
    embed1 = np.asarray(embed1, dtype=np.float32)
    embed2 = np.asarray(embed2, dtype=np.float32)
    in_proj_w = np.asarray(in_proj_w, dtype=np.float32)
    in_proj_b = np.asarray(in_proj_b, dtype=np.float32)
    W1 = np.asarray(W1, dtype=np.float32)
    b1 = np.asarray(b1, dtype=np.float32)
    W2 = np.asarray(W2, dtype=np.float32)
    b2 = np.asarray(b2, dtype=np.float32)

    nc = _get_program(int(os.environ.get("KERNEL_REPEAT", "1")))
    in_maps = make_in_maps(embed1, embed2, in_proj_w, in_proj_b, W1, b1, W2, b2)
    res = run_bass_kernel_spmd(nc, in_maps, list(range(NCORES)))
    return _assemble(res.results)


# revision 27
# speedup vs baseline: 6.1925x; 2.5519x over previous
"""Trainium2 Bass kernel for nn_CrossAttentionMatcher.

Reference math (B=2, L1=L2=512, H=256, NH=8, DH=32):
  q = e1 @ Wq.T + bq ; k = e2 @ Wk.T + bk
  logits[b,h,i,j] = (q.head[h][i] . k.head[h][j]) / sqrt(DH)
  attn_weights   = softmax(logits, -1).mean(heads)
  u = e1 @ W1a.T + b1 ; v = e2 @ W1b.T
  match_scores[b,i,j] = sigmoid( sum_h W2[h]*relu(u[b,i,h]+v[b,j,h]) + b2 )

Sharding: the 1024 (b,i) rows split into 8 slices of 128 rows, one per
NeuronCore (cores 0-3 -> batch 0, 4-7 -> batch 1). Each core holds full
embed2 for its batch plus all params, and computes its [128, 512] slice of
both outputs.

Per-core device program (Tile framework), all matmuls bf16:
  - prep matmuls produce transposed activations (contraction on partitions):
    qT,kT (bias folded, q pre-scaled), u=(h1T+b1) fp32, v=h2T bf16
  - attention: 8 K=32 matmuls -> PSUM logits; ACT exp with accum_out giving
    row sums for free; DVE reciprocal + ACT per-head scale + DVE strided
    head-sum
  - pairwise MLP (the bulk): for each of 128 i-rows x 2 h-blocks,
    R = relu(v + u[:,i]) as one DVE tensor_scalar (bf16 4x mode) or ACT
    activation (load-balance offload). R is the stationary matmul operand
    (4 j-blocks of [128,128]); w2 streams as N=1 moving operand; outputs
    scoresT[j, (jb,i)] accumulate into ONE PSUM bank as a single
    accumulation group (start only on the very first matmul -- start=True
    clears has_written for the whole bank; per-element has_written gives
    overwrite-on-first-touch then accumulate). Host transposes back.
  - sigmoid computed as exp + 1/(1+e) so every ACT op stays in the
    exp_and_others table set (one table load per kernel).

build_program(repeat=N) wraps the compute body in a tc.For_i loop for
wall-clock slope timing (the graded path uses repeat=1).
"""

import os
import numpy as np
import ml_dtypes

import concourse.bass as bass
import concourse.bacc as bacc
import concourse.mybir as mybir
from concourse import tile
from concourse.bass_utils import run_bass_kernel_spmd

F32 = mybir.dt.float32
BF16 = mybir.dt.bfloat16
AF = mybir.ActivationFunctionType
ALU = mybir.AluOpType
ET = mybir.EngineType

H = 256
NH = 8
DH = 32
B = 2
L1 = 512
L2 = 512
NCORES = 8
SL = 128          # L1-rows per core
CB = 2            # contraction blocks (256 = 2*128)
HB = 2            # hidden blocks (256 = 2*128)
JB = 4            # j blocks of 128
QSCALE = 1.0 / np.sqrt(DH)

# pairwise R-tile engine assignment by k%32 (GPSIMD measured ~8us/op on HW --
# never route compute there)
R_ACT_SLOTS = frozenset(
    int(x) for x in os.environ.get(
        "KERNEL_R_ACT", "1,6,11,16,21,26,30").split(",") if x)


def build_program(repeat=1):
    nc = bacc.Bacc(
        "TRN2",
        target_bir_lowering=False,
        debug=False,
    )

    # ---- DRAM I/O (per-core shards; names are the in_map keys) ----
    e1t_d = nc.dram_tensor("e1t", [CB, 128, SL], BF16, kind="ExternalInput")
    e2t_d = nc.dram_tensor("e2t", [CB, 128, L2], BF16, kind="ExternalInput")
    wqt_d = nc.dram_tensor("wqt", [CB, 128, H], BF16, kind="ExternalInput")
    wkt_d = nc.dram_tensor("wkt", [CB, 128, H], BF16, kind="ExternalInput")
    w1at_d = nc.dram_tensor("w1at", [CB, 128, H], BF16, kind="ExternalInput")
    w1bt_d = nc.dram_tensor("w1bt", [CB, 128, H], BF16, kind="ExternalInput")
    bq_d = nc.dram_tensor("bq", [128, HB], F32, kind="ExternalInput")
    bk_d = nc.dram_tensor("bk", [128, HB], F32, kind="ExternalInput")
    b1_d = nc.dram_tensor("b1c", [128, HB], F32, kind="ExternalInput")
    w2_d = nc.dram_tensor("w2c", [128, HB], BF16, kind="ExternalInput")
    b2n_d = nc.dram_tensor("b2neg", [128, 1], F32, kind="ExternalInput")
    attn_d = nc.dram_tensor("attn", [SL, L2], F32, kind="ExternalOutput")
    scores_d = nc.dram_tensor("scoresT", [128, JB, SL], F32,
                              kind="ExternalOutput")

    with tile.TileContext(nc) as tc:
        with (
            tc.tile_pool(name="const", bufs=1) as const,
            tc.tile_pool(name="work", bufs=1) as work,
            tc.tile_pool(name="rpool", bufs=64) as rpool,
            tc.tile_pool(name="pps", bufs=2, space="PSUM") as pps,
            tc.tile_pool(name="ppb", bufs=4, space="PSUM") as ppb,
            tc.tile_pool(name="pscp", bufs=1, space="PSUM") as pscp,
        ):
            # ---- loads (outside the repeat loop) ----
            E1T = const.tile([128, CB, SL], BF16, tag="e1t")
            E2T = const.tile([128, CB, L2], BF16, tag="e2t")
            WQ = const.tile([128, CB, H], BF16, tag="wqt")
            WK = const.tile([128, CB, H], BF16, tag="wkt")
            W1A = const.tile([128, CB, H], BF16, tag="w1at")
            W1B = const.tile([128, CB, H], BF16, tag="w1bt")
            BQ = const.tile([128, HB], F32, tag="bq")
            BK = const.tile([128, HB], F32, tag="bk")
            B1 = const.tile([128, HB], F32, tag="b1")
            W2C = const.tile([128, HB], BF16, tag="w2c")
            B2N = const.tile([128, 1], F32, tag="b2n")
            # u/v-path tensors first so prep matmuls start ASAP; spread across
            # the two HWDGE queues (SP + ACT)
            for cb in range(CB):
                nc.sync.dma_start(E2T[:, cb, :], e2t_d[cb])
                nc.scalar.dma_start(E1T[:, cb, :], e1t_d[cb])
                nc.scalar.dma_start(W1A[:, cb, :], w1at_d[cb])
                nc.sync.dma_start(W1B[:, cb, :], w1bt_d[cb])
            nc.scalar.dma_start(B1[:], b1_d[:])
            nc.scalar.dma_start(W2C[:], w2_d[:])
            for cb in range(CB):
                nc.scalar.dma_start(WQ[:, cb, :], wqt_d[cb])
                nc.sync.dma_start(WK[:, cb, :], wkt_d[cb])
            nc.scalar.dma_start(BQ[:], bq_d[:])
            nc.sync.dma_start(BK[:], bk_d[:])
            nc.sync.dma_start(B2N[:], b2n_d[:])

            U = work.tile([128, HB, SL], F32, tag="u")
            V = work.tile([128, HB, L2], BF16, tag="v")
            QT = work.tile([128, HB, SL], BF16, tag="qt")
            KT = work.tile([128, HB, L2], BF16, tag="kt")
            EXPS = work.tile([128, NH, L2], F32, tag="exps")
            S8 = work.tile([128, NH], F32, tag="s8")
            C8 = work.tile([128, NH], F32, tag="c8")
            ATTN = work.tile([128, L2], F32, tag="attn")
            SCE = work.tile([128, JB, SL], F32, tag="sce")
            SCORES = work.tile([128, JB, SL], F32, tag="scores")

            def emit_body():
                # ---- prep matmuls for the pairwise MLP (u, v) ----
                for m in range(HB):
                    hs = slice(m * 128, (m + 1) * 128)
                    pu = pps.tile([128, SL], F32, tag="pp_small")
                    for cb in range(CB):
                        nc.tensor.matmul(pu[:], W1A[:, cb, hs], E1T[:, cb, :],
                                         start=(cb == 0), stop=(cb == 1))
                    nc.vector.tensor_scalar(U[:, m, :], pu[:], B1[:, m:m + 1],
                                            None, ALU.add)
                    pv = ppb.tile([128, L2], F32, tag="pp_big")
                    for cb in range(CB):
                        nc.tensor.matmul(pv[:], W1B[:, cb, hs], E2T[:, cb, :],
                                         start=(cb == 0), stop=(cb == 1))
                    nc.vector.tensor_copy(V[:, m, :], pv[:])

                psc = pscp.tile([128, JB, SL], F32, tag="psc")

                # single accumulation group for the whole bank (see docstring)
                nmm = [0]
                nmm_total = SL * HB * JB

                def pair_step(i, hb):
                    r = rpool.tile([128, L2], BF16, tag="r")
                    k = i * HB + hb
                    if k % 32 in R_ACT_SLOTS:
                        nc.scalar.activation(r[:], V[:, hb, :], AF.Relu,
                                             bias=U[:, hb, i:i + 1])
                    else:
                        nc.vector.tensor_scalar(r[:], V[:, hb, :],
                                                U[:, hb, i:i + 1], 0.0,
                                                ALU.add, ALU.max)
                    for jb in range(JB):
                        nc.tensor.matmul(psc[:, jb, i:i + 1],
                                         r[:, 128 * jb:128 * (jb + 1)],
                                         W2C[:, hb:hb + 1],
                                         start=(nmm[0] == 0),
                                         stop=(nmm[0] == nmm_total - 1),
                                         skip_group_check=True)
                        nmm[0] += 1

                pair = [(i, hb) for i in range(SL) for hb in range(HB)]
                pos = [0]

                def emit_pair(n):
                    for i, hb in pair[pos[0]:pos[0] + n]:
                        pair_step(i, hb)
                    pos[0] += n

                emit_pair(32)

                # ---- attention: qT, kT ----
                for m in range(HB):
                    hs = slice(m * 128, (m + 1) * 128)
                    pq = pps.tile([128, SL], F32, tag="pp_small")
                    for cb in range(CB):
                        nc.tensor.matmul(pq[:], WQ[:, cb, hs], E1T[:, cb, :],
                                         start=(cb == 0), stop=(cb == 1))
                    # host pre-scales bq by QSCALE: qT = pq*QSCALE + bq*QSCALE
                    nc.vector.tensor_scalar(QT[:, m, :], pq[:], QSCALE,
                                            BQ[:, m:m + 1], ALU.mult, ALU.add)
                    pk = ppb.tile([128, L2], F32, tag="pp_big")
                    for cb in range(CB):
                        nc.tensor.matmul(pk[:], WK[:, cb, hs], E2T[:, cb, :],
                                         start=(cb == 0), stop=(cb == 1))
                    nc.vector.tensor_scalar(KT[:, m, :], pk[:], BK[:, m:m + 1],
                                            None, ALU.add)

                emit_pair(16)

                # ---- logits + exp (row-sums free via accum_out) ----
                # 4 heads per group use distinct PE row-groups and distinct
                # PSUM banks; issued back-to-back they overlap in the array
                for g in range(2):
                    pls = []
                    for j in range(4):
                        hd = g * 4 + j
                        m = hd // 4
                        ps = slice(32 * j, 32 * j + 32)
                        pl = ppb.tile([128, L2], F32, tag="pp_big",
                                      name=f"pl_{hd}")
                        nc.tensor.matmul(pl[:], QT[ps, m, :], KT[ps, m, :],
                                         start=True, stop=True,
                                         tile_position=(32 * j, 0))
                        pls.append(pl)
                    for j in range(4):
                        hd = g * 4 + j
                        nc.scalar.activation(EXPS[:, hd, :], pls[j][:], AF.Exp,
                                             accum_out=S8[:, hd:hd + 1])
                        emit_pair(8)

                # ---- softmax tail ----
                nc.vector.reciprocal(C8[:], S8[:])
                nc.vector.tensor_scalar(C8[:], C8[:], 1.0 / NH, None, ALU.mult)
                emit_pair(8)
                for hd in range(NH):
                    nc.scalar.activation(EXPS[:, hd, :], EXPS[:, hd, :],
                                         AF.Copy, scale=C8[:, hd:hd + 1])
                    emit_pair(4)
                emit_pair(16)
                # head-sum: strided view [128, j, h], reduce innermost h
                nc.vector.tensor_reduce(ATTN[:],
                                        EXPS[:].rearrange("p h j -> p j h"),
                                        axis=mybir.AxisListType.X, op=ALU.add)
                nc.sync.dma_start(attn_d[:], ATTN[:])

                # sigmoid(x+b2) = 1/(1+exp(-x-b2)); Exp keeps the ACT table
                # set unchanged (Sigmoid lives in a different set).  Emitted
                # in two i-halves so the first half's exp/add/recip/store
                # overlaps the second half's pairwise stream (costs one short
                # PE stall from the PSUM bank-read serialization, saves most
                # of the ~3us serial tail).
                def sigmoid_half(h):
                    isl = slice(64 * h, 64 * h + 64)
                    nc.scalar.activation(SCE[:, :, isl], psc[:, :, isl],
                                         AF.Exp, bias=B2N[:, 0:1], scale=-1.0)
                    nc.vector.tensor_scalar(SCE[:, :, isl], SCE[:, :, isl],
                                            1.0, None, ALU.add)
                    nc.vector.reciprocal(SCORES[:, :, isl], SCE[:, :, isl])
                    nc.sync.dma_start(scores_d[:, :2, isl],
                                      SCORES[:, :2, isl])
                    nc.scalar.dma_start(scores_d[:, 2:, isl],
                                        SCORES[:, 2:, isl])

                # all i < 64 columns of psc are final once pair index 128 has
                # been emitted; pos is past that here
                assert pos[0] >= 128
                sigmoid_half(0)
                emit_pair(len(pair) - pos[0])
                sigmoid_half(1)

            if repeat == 1:
                emit_body()
            else:
                with tc.For_i(0, repeat, 1,
                              hint_engines=(ET.PE, ET.DVE, ET.Activation)):
                    emit_body()

    nc.compile()
    return nc


_NC_CACHE = {}


def _get_program(repeat=1):
    if repeat not in _NC_CACHE:
        _NC_CACHE[repeat] = build_program(repeat)
    return _NC_CACHE[repeat]


def make_in_maps(embed1, embed2, in_proj_w, in_proj_b, W1, b1, W2, b2):
    bf = ml_dtypes.bfloat16
    f32 = np.float32

    def blkT(a):  # [n, c] -> transposed + contraction-blocked [CB, 128, n]
        return np.ascontiguousarray(
            a.astype(f32).T.reshape(CB, 128, -1)).astype(bf)

    wqt = blkT(in_proj_w[:H])
    wkt = blkT(in_proj_w[H:2 * H])
    w1at = blkT(W1[:, :H])       # [h, c] rows x contraction cols
    w1bt = blkT(W1[:, H:])
    # pre-scaled: device computes qT = (pq + bq)*QSCALE as pq*QSCALE + this
    bq = np.ascontiguousarray(
        in_proj_b[:H].reshape(HB, 128).T * QSCALE).astype(f32)
    bk = np.ascontiguousarray(in_proj_b[H:2 * H].reshape(HB, 128).T).astype(f32)
    b1c = np.ascontiguousarray(b1.reshape(HB, 128).T).astype(f32)
    w2c = np.ascontiguousarray(W2[0].reshape(HB, 128).T).astype(bf)
    b2neg = np.full((128, 1), -b2[0], dtype=f32)

    in_maps = []
    for c in range(NCORES):
        b, s = divmod(c, NCORES // B)
        r0 = s * SL
        in_maps.append({
            "e1t": blkT(embed1[b, r0:r0 + SL]),
            "e2t": blkT(embed2[b]),
            "wqt": wqt, "wkt": wkt, "w1at": w1at, "w1bt": w1bt,
            "bq": bq, "bk": bk, "b1c": b1c, "w2c": w2c, "b2neg": b2neg,
        })
    return in_maps


def _assemble(results):
    attn = np.empty((B, L1, L2), dtype=np.float32)
    scores = np.empty((B, L1, L2), dtype=np.float32)
    for c in range(NCORES):
        b, s = divmod(c, NCORES // B)
        r0 = s * SL
        attn[b, r0:r0 + SL] = results[c]["attn"].astype(np.float32)
        # scoresT is [j_in, jb, i] -> [i, jb*128 + j_in]
        st = results[c]["scoresT"].astype(np.float32)
        scores[b, r0:r0 + SL] = np.transpose(st, (2, 1, 0)).reshape(SL, L2)
    return attn, scores


def kernel(embed1, embed2, in_proj_w, in_proj_b, W1, b1, W2, b2):
    embed1 = np.asarray(embed1, dtype=np.float32)
    embed2 = np.asarray(embed2, dtype=np.float32)
    in_proj_w = np.asarray(in_proj_w, dtype=np.float32)
    in_proj_b = np.asarray(in_proj_b, dtype=np.float32)
    W1 = np.asarray(W1, dtype=np.float32)
    b1 = np.asarray(b1, dtype=np.float32)
    W2 = np.asarray(W2, dtype=np.float32)
    b2 = np.asarray(b2, dtype=np.float32)

    nc = _get_program(int(os.environ.get("KERNEL_REPEAT", "1")))
    in_maps = make_in_maps(embed1, embed2, in_proj_w, in_proj_b, W1, b1, W2, b2)
    res = run_bass_kernel_spmd(nc, in_maps, list(range(NCORES)))
    return _assemble(res.results)
